# revision 1
# baseline (speedup 1.0000x reference)
"""Trainium2 Bass kernel for nn_Attention (dense transformer block):
y = Attention(RoPE(x@wqT), RoPE(x@wkT), x@wvT, causal) @ woT

Sharding: 8 cores = 2 batches x 4 head-groups (tensor-parallel heads,
data-parallel batch).  Each core handles one batch and 4 of the 16 heads
(512 of the 2048 channels): column-shard of wq/wk/wv, row-shard of wo.
Each core emits a full-shape [S, D] partial of y; the host sums the 4
partials per batch.

Kernel layout strategy (per core, SPMD — identical program, per-core data):
  - qT/kT computed directly in [head_dim, seq] layout (lhsT = wqT slice,
    rhs = xT streamed from DRAM).  RoPE pair-swap done with a DVE
    stream_shuffle + DVE/GpSimd combine against sign-folded cos/sin tables.
  - v computed in natural [seq, head_dim] layout (lhsT = xV column block,
    a host-retiled copy of x that makes those loads contiguous).
  - scores computed transposed: sT[sk, sq] = kT_tile.T @ qT_chunk, so the
    PV matmul needs no transposes.  Softmax runs without max subtraction
    (scores are bounded, |s*scale| < ~6); denominators by summing the prob
    tiles elementwise on the otherwise-idle GpSimd engine (final add on DVE
    to produce the fp32r tag), then a single all-ones [128,128] stationary
    matmul per chunk reduces over partitions and leaves the sum broadcast.
  - causal masking: off-diagonal upper tiles skipped entirely; the 4
    diagonal-straddling tile shapes multiply post-exp by host-built 0/1
    masks.
  - all matmuls run as float32r (full-rate fp32 path on the PE).
"""

import os
import sys

import numpy as np

for _p in ("/opt/trn_rl_repo", "/root/.axon_site/_ro/trn_rl_repo"):
    if os.path.isdir(_p) and _p not in sys.path:
        sys.path.insert(0, _p)

import concourse.bass as bass
import concourse.tile as tile
from concourse import bacc
from concourse import mybir
from concourse import bass_utils

B, S, D, H = 2, 2048, 2048, 16
HD = 128                 # head dim
HPC = 4                  # heads per core
CPB = 4                  # cores per batch
N_CORES = 8
NK = D // 128            # 16 contraction chunks
NSQ = S // 512           # 4 sq chunks of 512
NSK = S // 128           # 16 sk tiles of 128
SCALE = float(1.0 / np.sqrt(np.float32(HD)))

F32 = mybir.dt.float32
F32R = mybir.dt.float32r
USE_F32R = True

EXP = mybir.ActivationFunctionType.Exp
SWAP_MASK = [i ^ 1 for i in range(32)]


MMDT = F32R if USE_F32R else F32


def round_fp32r(x):
    """Round fp32 array to fp32r (e8m11) with round-to-nearest-even."""
    if not USE_F32R:
        return np.ascontiguousarray(x, dtype=np.float32)
    v = np.ascontiguousarray(x, np.float32).view(np.uint32)
    b = (v >> 12) & 1
    v = (v + 0x7FF + b) & np.uint32(0xFFFFF000)
    return v.view(np.float32)


def _emit(tc):
    nc = tc.nc

    xT = nc.dram_tensor("xT", [D, S], MMDT, kind="ExternalInput").ap()
    xV = nc.dram_tensor("xV", [S, D], MMDT, kind="ExternalInput").ap()
    wqT = nc.dram_tensor("wqT", [D, HPC * HD], MMDT, kind="ExternalInput").ap()
    wkT = nc.dram_tensor("wkT", [D, HPC * HD], MMDT, kind="ExternalInput").ap()
    wvT = nc.dram_tensor("wvT", [D, HPC * HD], MMDT, kind="ExternalInput").ap()
    woT = nc.dram_tensor("woT", [HPC * HD, D], MMDT, kind="ExternalInput").ap()
    cosq = nc.dram_tensor("cosq", [HD, S], F32, kind="ExternalInput").ap()
    sinq = nc.dram_tensor("sinq", [HD, S], F32, kind="ExternalInput").ap()
    dmask = nc.dram_tensor("dmask", [4, 128, 512], MMDT, kind="ExternalInput").ap()
    onesd = nc.dram_tensor("onesd", [128, 128], MMDT, kind="ExternalInput").ap()
    y = nc.dram_tensor("y", [S, D], F32, kind="ExternalOutput").ap()

    # two DMA issue queues: SP for the latency-critical stream, ACT for the rest
    dma_a = nc.sync
    dma_b = nc.scalar

    # long-lived pools first (stack allocator wants LIFO release order)
    consts = tc.alloc_tile_pool(name="consts", bufs=1)
    qk_pool = tc.alloc_tile_pool(name="qkp", bufs=HPC)
    qT = [qk_pool.tile([128, S], MMDT, name=f"qT{h}", tag="qT") for h in range(HPC)]
    kT = [qk_pool.tile([128, S], MMDT, name=f"kT{h}", tag="kT") for h in range(HPC)]

    # ---- phase 1a (merged): q and k projections (+RoPE), single x pass
    ones_sq = consts.tile([128, 128], MMDT, name="ones_sq")
    mask_sb = []
    for m in range(4):
        mt = consts.tile([128, 512], MMDT, name=f"mask{m}", tag=f"mask{m}")
        mask_sb.append(mt)
    ropec = tc.alloc_tile_pool(name="ropec", bufs=1)
    cos_sb = ropec.tile([128, S], F32, name="cos_sb")
    sin_sb = ropec.tile([128, S], F32, name="sin_sb")
    tpool = tc.alloc_tile_pool(name="tqk", bufs=2)

    wk_pool = tc.alloc_tile_pool(name="wkp", bufs=NK)
    xpool = tc.alloc_tile_pool(name="xqk", bufs=NK + 2)
    wq_pool = tc.alloc_tile_pool(name="wqp", bufs=NK)
    pspool = tc.alloc_tile_pool(name="psqk", bufs=8, space="PSUM")

    # interleave weight and first-chunk x loads so the k-loop starts early
    wq_sb, wk_sb, xs0 = [], [], []
    for k in range(NK):
        wt = wq_pool.tile([128, HPC * HD], MMDT, name=f"wq{k}", tag="wq")
        dma_a.dma_start(out=wt, in_=wqT[128 * k:128 * (k + 1), :])
        wq_sb.append(wt)
        xt = xpool.tile([128, 512], MMDT, name=f"x_0_{k}", tag="xs")
        eng = dma_b if k % 2 == 0 else dma_a
        eng.dma_start(out=xt, in_=xT[128 * k:128 * (k + 1), 0:512])
        xs0.append(xt)
        wt = wk_pool.tile([128, HPC * HD], MMDT, name=f"wk{k}", tag="wk")
        dma_b.dma_start(out=wt, in_=wkT[128 * k:128 * (k + 1), :])
        wk_sb.append(wt)
    # rope/mask constants arrive behind the first chunk's stream
    dma_b.dma_start(out=cos_sb, in_=cosq)
    dma_b.dma_start(out=sin_sb, in_=sinq)
    dma_b.dma_start(out=ones_sq, in_=onesd)
    for m in range(4):
        dma_b.dma_start(out=mask_sb[m], in_=dmask[m])


    for j in range(NSQ):
        sl = slice(512 * j, 512 * (j + 1))
        if j == 0:
            xs = xs0
        else:
            xs = []
            for k in range(NK):
                xt = xpool.tile([128, 512], MMDT, name=f"x_{j}_{k}", tag="xs")
                eng = dma_a if k % 2 == 0 else dma_b
                eng.dma_start(out=xt, in_=xT[128 * k:128 * (k + 1), sl])
                xs.append(xt)
        for w_sb, dsts, tagn in ((wq_sb, qT, "q"), (wk_sb, kT, "k")):
            accs = [
                pspool.tile(
                    [128, 512], F32, name=f"acc{tagn}_{j}_{h}", tag="acc"
                )
                for h in range(HPC)
            ]
            for k in range(NK):
                for h in range(HPC):
                    nc.tensor.matmul(
                        accs[h], w_sb[k][:, 128 * h:128 * (h + 1)], xs[k],
                        start=(k == 0), stop=(k == NK - 1),
                    )
            for h in range(HPC):
                acc, dst = accs[h], dsts[h]
                raw = tpool.tile([128, 512], F32, name=f"raw{tagn}_{j}_{h}", tag="raw")
                nc.vector.tensor_copy(out=raw, in_=acc)
                shuf = tpool.tile([128, 512], F32, name=f"sh{tagn}_{j}_{h}", tag="shuf")
                nc.vector.stream_shuffle(shuf, acc, SWAP_MASK)
                t1 = tpool.tile([128, 512], F32, name=f"t1{tagn}_{j}_{h}", tag="t1")
                nc.vector.tensor_mul(t1, shuf, sin_sb[:, sl])
                t2 = tpool.tile([128, 512], F32, name=f"t2{tagn}_{j}_{h}", tag="t2")
                nc.gpsimd.tensor_mul(t2, raw, cos_sb[:, sl])
                nc.vector.tensor_add(dst[:, sl], t1, t2)
    wq_pool.release()

    # ---- phase 1b: v projection in natural [seq, head_dim] layout
    # wv tiles recycle the wk pool's slots; accv tiles recycle the ph1a psum
    # tag — both avoid pool-boundary serialization at the phase seam.
    wv_sb = []
    for k in range(NK):
        wt = wk_pool.tile([128, HPC * HD], MMDT, name=f"wv{k}", tag="wk")
        dma_b.dma_start(out=wt, in_=wvT[128 * k:128 * (k + 1), :])
        wv_sb.append(wt)

    v_pool = tc.alloc_tile_pool(name="vp", bufs=NSK, side="right")
    v_sb = [v_pool.tile([128, HPC * HD], MMDT, name=f"v{m}", tag="v") for m in range(NSK)]
    for m in range(NSK):
        xcp = []
        for g in range(4):
            xt = xpool.tile([128, 4, 128], MMDT, name=f"xc{m}_{g}", tag="xs")
            eng = dma_a if g % 2 == 0 else dma_b
            eng.dma_start(
                out=xt,
                in_=xV[128 * m:128 * (m + 1), 512 * g:512 * (g + 1)].rearrange(
                    "p (kt c) -> p kt c", c=128
                ),
            )
            xcp.append(xt)
        acc = pspool.tile([128, HPC * HD], F32, name=f"accv{m}", tag="acc")
        for k in range(NK):
            nc.tensor.matmul(
                acc, xcp[k // 4][:, k % 4, :], wv_sb[k],
                start=(k == 0), stop=(k == NK - 1),
            )
        nc.vector.tensor_copy(out=v_sb[m], in_=acc)
    xpool.release()
    pspool.release()
    wk_pool.release()
    tpool.release()
    ropec.release()

    # ---- phase 2: causal attention per head, transposed-score layout
    oh_pool = tc.alloc_tile_pool(name="ohp", bufs=HPC, side="right")
    out_hT = [oh_pool.tile([128, S], MMDT, name=f"oh{h}", tag="oh") for h in range(HPC)]
    # prefetch wo during attention
    wo_pool = tc.alloc_tile_pool(name="wop", bufs=HPC, side="right")
    wo_sb = []
    for h in range(HPC):
        wt = wo_pool.tile([128, D], MMDT, name=f"wo{h}", tag="wo")
        dma_a.dma_start(out=wt, in_=woT[128 * h:128 * (h + 1), :])
        wo_sb.append(wt)

    pp = tc.alloc_tile_pool(name="pp", bufs=8)
    small2 = tc.alloc_tile_pool(name="small2", bufs=4)
    pss = tc.alloc_tile_pool(name="pss", bufs=5, space="PSUM")
    psd = tc.alloc_tile_pool(name="psd", bufs=1, space="PSUM")
    pspv = tc.alloc_tile_pool(name="pspv", bufs=2, space="PSUM")
    for h in range(HPC):
        for j in range(NSQ):
            sl = slice(512 * j, 512 * (j + 1))
            nsk = 4 * j + 4
            den = psd.tile([128, 512], F32, name=f"den{h}_{j}", tag="den")
            pv = pspv.tile([128, 512], F32, name=f"pv{h}_{j}", tag="pv")
            # diagonal (masked) tiles first: their exp->mask latency hides
            # under the unmasked tiles' pv matmuls.  The denominator is the
            # elementwise sum of all pt tiles (GpSimd running adds, last add
            # on DVE to produce the fp32r tag) reduced by ONE ones-matmul.
            order = list(range(4 * j, nsk)) + list(range(0, 4 * j))
            # diagonal tile with mask pattern m: columns sql < 128*m are
            # fully masked — compute only a column slice (kept >= 256 wide
            # so fp32r stays at full rate; m=3 pays 128 wasted columns)
            offs = {0: 0, 1: 128, 2: 256, 3: 256}
            pacc = None
            pts = []
            for idx, i in enumerate(order):
                off = offs[i - 4 * j] if i >= 4 * j else 0
                cs = slice(off, 512)
                qs = slice(512 * j + off, 512 * (j + 1))
                s_ps = pss.tile([128, 512], F32, name=f"s{h}_{j}_{i}", tag="s")
                nc.tensor.matmul(
                    s_ps[:, cs], kT[h][:, 128 * i:128 * (i + 1)], qT[h][:, qs],
                    start=True, stop=True,
                )
                pt = pp.tile([128, 512], MMDT, name=f"p{h}_{j}_{i}", tag="pt")
                nc.scalar.activation(pt[:, cs], s_ps[:, cs], EXP, bias=0.0, scale=SCALE)
                if i >= 4 * j:
                    nc.vector.tensor_mul(pt[:, cs], pt[:, cs], mask_sb[i - 4 * j][:, cs])
                nc.tensor.matmul(
                    pv[:, cs], v_sb[i][:, 128 * h:128 * (h + 1)], pt[:, cs],
                    start=(idx == 0), stop=(idx == nsk - 1),
                )
                pts.append((pt, off))
                if idx == 1:
                    pacc = small2.tile(
                        [128, 512], F32, name=f"pa{h}_{j}", tag="pacc", bufs=2
                    )
                    nc.gpsimd.tensor_copy(out=pacc, in_=pts[0][0])
                    o1 = pts[1][1]
                    nc.gpsimd.tensor_add(
                        pacc[:, o1:], pacc[:, o1:], pt[:, o1:]
                    )
                elif 1 < idx < nsk - 1:
                    nc.gpsimd.tensor_add(pacc[:, off:], pacc[:, off:], pt[:, cs])
                elif idx == nsk - 1:
                    pacc_r = small2.tile(
                        [128, 512], MMDT, name=f"par{h}_{j}", tag="paccr", bufs=2
                    )
                    if off > 0:
                        nc.vector.tensor_copy(out=pacc_r[:, 0:off], in_=pacc[:, 0:off])
                    nc.vector.tensor_add(
                        pacc_r[:, cs], pacc[:, cs], pt[:, cs]
                    )
                    nc.tensor.matmul(den, ones_sq, pacc_r, start=True, stop=True)
            recip = small2.tile([128, 512], F32, name=f"rc{h}_{j}", tag="recip")
            nc.vector.reciprocal(recip, den)
            nc.vector.tensor_mul(out_hT[h][:, sl], pv, recip)
    pspv.release()
    psd.release()
    pss.release()
    small2.release()
    pp.release()
    qk_pool.release()

    # ---- phase 3: row-parallel wo partial product, row-block output DMAs
    ys_pool = tc.alloc_tile_pool(name="ysp", bufs=3)
    psy_pool = tc.alloc_tile_pool(name="psy", bufs=3, space="PSUM")
    for t in range(NSK):
        ys = ys_pool.tile([128, D], F32, name=f"ys{t}", tag="ys")
        for n in range(NSQ):
            acc = psy_pool.tile([128, 512], F32, name=f"accy{t}_{n}", tag="y")
            for h in range(HPC):
                nc.tensor.matmul(
                    acc,
                    out_hT[h][:, 128 * t:128 * (t + 1)],
                    wo_sb[h][:, 512 * n:512 * (n + 1)],
                    start=(h == 0),
                    stop=(h == HPC - 1),
                )
            nc.vector.tensor_copy(out=ys[:, 512 * n:512 * (n + 1)], in_=acc)
        dma_a.dma_start(out=y[128 * t:128 * (t + 1), :], in_=ys)
    psy_pool.release()
    ys_pool.release()
    wo_pool.release()
    oh_pool.release()
    v_pool.release()
    consts.release()


_PROGRAM = None


def build_program():
    global _PROGRAM
    if _PROGRAM is None:
        nc = bacc.Bacc("TRN2", target_bir_lowering=False, debug=False)
        with tile.TileContext(nc) as tc:
            _emit(tc)
        nc.compile()
        _PROGRAM = nc
    return _PROGRAM


def make_core_inputs(x, freqs_cos, freqs_sin, wq, wk, wv, wo):
    """Host-side sharding: returns list of 8 per-core input dicts."""
    x = np.asarray(x, dtype=np.float32)
    freqs_cos = np.asarray(freqs_cos, dtype=np.float32)
    freqs_sin = np.asarray(freqs_sin, dtype=np.float32)
    wq = np.asarray(wq, dtype=np.float32)
    wk = np.asarray(wk, dtype=np.float32)
    wv = np.asarray(wv, dtype=np.float32)
    wo = np.asarray(wo, dtype=np.float32)

    cosq = np.ascontiguousarray(np.repeat(freqs_cos.T, 2, axis=0))  # [128, S]
    sinq = np.ascontiguousarray(np.repeat(freqs_sin.T, 2, axis=0))
    sinq[0::2, :] *= -1.0  # even rows: -sin; odd rows: +sin

    skl = np.arange(128)[:, None]
    sql = np.arange(512)[None, :]
    dmask = np.stack(
        [(128 * m + skl <= sql).astype(np.float32) for m in range(4)]
    )  # [4, 128, 512]

    onesd = np.ones((128, 128), dtype=np.float32)
    xTs = [round_fp32r(x[b].T) for b in range(B)]
    # V-phase layout: xV[128m+p, 128kt+c] = x[b][128m+c, 128kt+p]
    xVs = [
        np.ascontiguousarray(
            xr.T.reshape(16, 128, 16, 128).transpose(0, 3, 2, 1).reshape(2048, 2048)
        )
        for xr in xTs
    ]
    in_maps = []
    for c in range(N_CORES):
        b, g = divmod(c, CPB)
        hsl = slice(512 * g, 512 * (g + 1))
        in_maps.append(
            {
                "xT": xTs[b],
                "xV": xVs[b],
                "wqT": round_fp32r(wq[hsl, :].T),
                "wkT": round_fp32r(wk[hsl, :].T),
                "wvT": round_fp32r(wv[hsl, :].T),
                "woT": round_fp32r(wo[:, hsl].T),
                "cosq": cosq,
                "sinq": sinq,
                "dmask": dmask,
                "onesd": onesd,
            }
        )
    return in_maps


def run(inputs, trace=False, **spmd_kwargs):
    """Run the SPMD kernel on 8 cores.  Returns (y_full, BassKernelResults)."""
    nc = build_program()
    in_maps = make_core_inputs(
        inputs["x"], inputs["freqs_cos"], inputs["freqs_sin"],
        inputs["wq"], inputs["wk"], inputs["wv"], inputs["wo"],
    )
    res = bass_utils.run_bass_kernel_spmd(
        nc, in_maps, list(range(N_CORES)), trace=trace, **spmd_kwargs
    )
    out = np.zeros((B, S, D), dtype=np.float32)
    for c in range(N_CORES):
        out[c // CPB] += res.results[c]["y"]
    return out, res


def kernel(**inputs):
    out, _ = run(inputs, trace=False)
    return out


def simulate_core(core_idx, inputs):
    """CoreSim-validate a single core's program; returns its partial y."""
    from concourse.bass_interp import CoreSim

    nc = build_program()
    in_maps = make_core_inputs(
        inputs["x"], inputs["freqs_cos"], inputs["freqs_sin"],
        inputs["wq"], inputs["wk"], inputs["wv"], inputs["wo"],
    )
    sim = CoreSim(nc)
    for name, arr in in_maps[core_idx].items():
        sim.tensor(name)[:] = arr
    sim.simulate()
    return np.array(sim.tensor("y"))



# revision 7
# speedup vs baseline: 1.0902x; 1.0902x over previous
"""Trainium2 Bass kernel for nn_Attention (dense transformer block):
y = Attention(RoPE(x@wqT), RoPE(x@wkT), x@wvT, causal) @ woT

Sharding: 8 cores = 2 batches x 4 head-groups (tensor-parallel heads,
data-parallel batch).  Each core handles one batch and 4 of the 16 heads
(512 of the 2048 channels): column-shard of wq/wk/wv, row-shard of wo.
Each core emits a full-shape [S, D] partial of y; the host sums the 4
partials per batch.

Fused chunk pipeline (per core, SPMD): for each 512-row seq chunk j,
  proj(j):  q/k (+RoPE) and v for chunk j from one pass over x tiles
            (v reuses the same SBUF x tiles as stationary operands —
            x is streamed from DRAM exactly once, in bf16)
  attn(j):  causal attention for all 4 heads over keys 0..j, transposed
            scores (sT[sk,sq] = kT.T @ qT), exp on ACT, causal mask folded
            into the score psum via an identity-matmul of a -1e4 pattern,
            denominator via accumulating ones-matmul on the PE, p/v in
            bf16, software-pipelined with a 2-tile lookahead so the PE
            never waits on exp
  wo(j):    row-parallel wo partial for the chunk's 4 row-blocks, DMA'd
            out per row-block
Chunk j+1's x tiles prefetch during attn(j); all weights stay resident.
"""

import os
import sys

import numpy as np

for _p in ("/opt/trn_rl_repo", "/root/.axon_site/_ro/trn_rl_repo"):
    if os.path.isdir(_p) and _p not in sys.path:
        sys.path.insert(0, _p)

import concourse.bass as bass
import concourse.tile as tile
from concourse import bacc
from concourse import mybir
from concourse import bass_utils

B, S, D, H = 2, 2048, 2048, 16
HD = 128                 # head dim
HPC = 4                  # heads per core
CPB = 4                  # cores per batch
N_CORES = 8
NK = D // 128            # 16 contraction chunks
NSQ = S // 512           # 4 sq chunks of 512
NSK = S // 128           # 16 sk tiles of 128
SCALE = float(1.0 / np.sqrt(np.float32(HD)))

F32 = mybir.dt.float32
F32R = mybir.dt.float32r
BF16 = mybir.dt.bfloat16

EXP = mybir.ActivationFunctionType.Exp
SWAP_MASK = [i ^ 1 for i in range(32)]

# diag-tile mask pattern m: columns < 128*m fully masked; keep score width
# >= 256 so the fp32r moving operand stays at full rate
DIAG_OFF = {0: 0, 1: 128, 2: 256, 3: 256}
LOOKAHEAD = 2


def _emit(tc):
    nc = tc.nc

    xT = nc.dram_tensor("xT", [D, S], BF16, kind="ExternalInput").ap()
    wqT = nc.dram_tensor("wqT", [D, HPC * HD], BF16, kind="ExternalInput").ap()
    wkT = nc.dram_tensor("wkT", [D, HPC * HD], BF16, kind="ExternalInput").ap()
    wvT = nc.dram_tensor("wvT", [D, HPC * HD], BF16, kind="ExternalInput").ap()
    woT = nc.dram_tensor("woT", [HPC * HD, D], BF16, kind="ExternalInput").ap()
    cosq = nc.dram_tensor("cosq", [HD, S], F32, kind="ExternalInput").ap()
    sinq = nc.dram_tensor("sinq", [HD, S], F32, kind="ExternalInput").ap()
    negm = nc.dram_tensor("negm", [128, 4, 512], BF16, kind="ExternalInput").ap()
    ident = nc.dram_tensor("ident", [128, 128], BF16, kind="ExternalInput").ap()
    onesd = nc.dram_tensor("onesd", [128, 128], BF16, kind="ExternalInput").ap()
    y = nc.dram_tensor("y", [S, D], BF16, kind="ExternalOutput").ap()

    ld = nc.sync        # all loads on the SP HWDGE queue
    st = nc.scalar      # y stores on the ACT HWDGE queue

    # ---- SBUF pools (all live for the whole kernel)
    consts = tc.alloc_tile_pool(name="consts", bufs=1)
    ident_sb = consts.tile([128, 128], BF16, name="ident_sb")
    ones_sb = consts.tile([128, 128], BF16, name="ones_sb")
    negm_sb = consts.tile([128, 4, 512], BF16, name="negm_sb")

    wpool = tc.alloc_tile_pool(name="wpool", bufs=1)
    wq_sb = wpool.tile([128, NK, HPC * HD], BF16, name="wq_sb")
    wk_sb = wpool.tile([128, NK, HPC * HD], BF16, name="wk_sb")
    wv_sb = wpool.tile([128, NK, HPC * HD], BF16, name="wv_sb")
    wo_sb = wpool.tile([128, HPC, D], BF16, name="wo_sb")

    kpool = tc.alloc_tile_pool(name="kpool", bufs=1)
    kT = [[kpool.tile([128, 512], F32R, name=f"kT{h}_{j}") for j in range(NSQ)]
          for h in range(HPC)]
    vpool = tc.alloc_tile_pool(name="vpool", bufs=1)
    v_sb = [vpool.tile([128, HPC * HD], BF16, name=f"v{m}") for m in range(NSK)]

    qpool = tc.alloc_tile_pool(name="qpool", bufs=1)
    ropec = tc.alloc_tile_pool(name="ropec", bufs=2)
    xpool = tc.alloc_tile_pool(name="xpool", bufs=2)
    tpool = tc.alloc_tile_pool(name="tpool", bufs=2)
    ptpool = tc.alloc_tile_pool(name="ptpool", bufs=5)
    opool = tc.alloc_tile_pool(name="opool", bufs=1)
    rpool = tc.alloc_tile_pool(name="rpool", bufs=2)
    ypool = tc.alloc_tile_pool(name="ypool", bufs=2)

    pg = tc.alloc_tile_pool(name="pg", bufs=2, space="PSUM")
    pss = tc.alloc_tile_pool(name="pss", bufs=LOOKAHEAD + 1, space="PSUM")
    pspv = tc.alloc_tile_pool(name="pspv", bufs=2, space="PSUM")
    psden = tc.alloc_tile_pool(name="psden", bufs=1, space="PSUM")

    # ---- prologue DMAs; xs/wq split in halves so the first chain starts
    # after ~1/4 of the lead-in bytes.  Loads split over both HWDGE queues.
    def load_x(j):
        xt = xpool.tile([128, NK, 512], BF16, name=f"xs{j}", tag="xs")
        src = xT[:, 512 * j:512 * (j + 1)].rearrange("(kt p) c -> p kt c", p=128)
        ld.dma_start(out=xt[:, 0:NK // 2, :], in_=src[:, 0:NK // 2, :])
        ld.dma_start(out=xt[:, NK // 2:, :], in_=src[:, NK // 2:, :])
        return xt

    def load_rope(j):
        ct = ropec.tile([128, 512], F32, name=f"cos{j}", tag="cos")
        st.dma_start(out=ct, in_=cosq[:, 512 * j:512 * (j + 1)])
        sn = ropec.tile([128, 512], F32, name=f"sin{j}", tag="sin")
        st.dma_start(out=sn, in_=sinq[:, 512 * j:512 * (j + 1)])
        return ct, sn

    xs = load_x(0)
    wq_src = wqT.rearrange("(kt p) c -> p kt c", p=128)
    st.dma_start(out=wq_sb[:, 0:NK // 2, :], in_=wq_src[:, 0:NK // 2, :])
    st.dma_start(out=wq_sb[:, NK // 2:, :], in_=wq_src[:, NK // 2:, :])
    cs0 = load_rope(0)
    ld.dma_start(out=wk_sb, in_=wkT.rearrange("(kt p) c -> p kt c", p=128))
    ld.dma_start(out=wv_sb, in_=wvT.rearrange("(kt p) c -> p kt c", p=128))
    st.dma_start(out=ident_sb, in_=ident)
    st.dma_start(out=ones_sb, in_=onesd)
    st.dma_start(out=negm_sb, in_=negm)
    ld.dma_start(out=wo_sb, in_=woT.rearrange("(h p) d -> p h d", p=128))

    qT = [None] * HPC    # per-chunk q tiles, rewritten each chunk
    out_h = [None] * HPC

    def proj_chains(j, xs, cos_sb, sin_sb):
        """Closures: 8 q/k chains (+RoPE drain on DVE), 4 v chains (ACT
        drain).  The psum is freed by the 2nd DVE op (t2) for q/k, and by
        the single ACT copy for v."""
        chains = []

        def qk_chain(which, w_sb, h):
            def emit():
                acc = pg.tile([128, 512], F32, name=f"a{which}{j}_{h}", tag="pg")
                for k in range(NK):
                    nc.tensor.matmul(
                        acc, w_sb[:, k, 128 * h:128 * (h + 1)], xs[:, k, :],
                        start=(k == 0), stop=(k == NK - 1),
                    )
                if which == "q":
                    dst = qpool.tile([128, 512], F32R, name=f"qT{h}_{j}", tag=f"q{h}")
                    qT[h] = dst
                else:
                    dst = kT[h][j]
                shuf = tpool.tile([128, 512], F32, name=f"sh{which}{j}_{h}", tag="shuf")
                nc.vector.stream_shuffle(shuf, acc, SWAP_MASK)
                t2 = tpool.tile([128, 512], F32, name=f"t2{which}{j}_{h}", tag="t1")
                nc.vector.tensor_mul(t2, acc, cos_sb)
                t1 = tpool.tile([128, 512], F32, name=f"t1{which}{j}_{h}", tag="shuf")
                nc.vector.tensor_mul(t1, shuf, sin_sb)
                nc.vector.tensor_add(dst, t1, t2)
            return emit

        def v_chain(m):
            def emit():
                acc = pg.tile([128, HPC * HD], F32, name=f"av{j}_{m}", tag="pg")
                for k in range(NK):
                    nc.tensor.matmul(
                        acc, xs[:, k, 128 * m:128 * (m + 1)], wv_sb[:, k, :],
                        start=(k == 0), stop=(k == NK - 1),
                    )
                nc.scalar.copy(v_sb[4 * j + m], acc)
            return emit

        for which, w_sb in (("q", wq_sb), ("k", wk_sb)):
            for h in range(HPC):
                chains.append(qk_chain(which, w_sb, h))
        for m in range(4):
            chains.append(v_chain(m))
        return chains

    def emit_attn(j):
        """Causal attention for all heads over keys 0..4j+3, pipelined."""
        nsk = 4 * j + 4
        # flat tile list: per head, off-diagonal tiles first, then diagonal
        tiles = []
        for h in range(HPC):
            order = list(range(0, 4 * j)) + list(range(4 * j, nsk))
            for pos, i in enumerate(order):
                tiles.append((h, i, pos == 0, pos == nsk - 1))
        n = len(tiles)
        st_of = {}   # tile idx -> (s_ps, pt, off)
        pv_of = {}
        den_of = {}

        def emit_score(t):
            h, i, first, last = tiles[t]
            off = DIAG_OFF[i - 4 * j] if i >= 4 * j else 0
            cs = slice(off, 512)
            s_ps = pss.tile([128, 512], F32, name=f"s{j}_{t}", tag="s")
            diag = i >= 4 * j
            nc.tensor.matmul(
                s_ps[:, cs], kT[h][i // 4][:, 128 * (i % 4):128 * (i % 4 + 1)],
                qT[h][:, cs], start=True, stop=not diag,
            )
            if diag:
                nc.tensor.matmul(
                    s_ps[:, cs], ident_sb, negm_sb[:, i - 4 * j, cs],
                    start=False, stop=True,
                )
            pt = ptpool.tile([128, 512], BF16, name=f"p{j}_{t}", tag="pt")
            nc.scalar.activation(pt[:, cs], s_ps[:, cs], EXP, bias=0.0, scale=SCALE)
            st_of[t] = (pt, off)

        def emit_pv(t):
            h, i, first, last = tiles[t]
            pt, off = st_of.pop(t)
            cs = slice(off, 512)
            if first:
                pv_of[h] = pspv.tile([128, 512], F32, name=f"pv{j}_{h}", tag="pv")
                den_of[h] = psden.tile([128, 512], F32, name=f"dn{j}_{h}", tag="den")
            nc.tensor.matmul(
                pv_of[h][:, cs], v_sb[i][:, 128 * h:128 * (h + 1)], pt[:, cs],
                start=first, stop=last,
            )
            nc.tensor.matmul(
                den_of[h][:, cs], ones_sb, pt[:, cs],
                start=first, stop=last,
            )
            if last:
                recip = rpool.tile([128, 512], F32, name=f"rc{j}_{h}", tag="recip")
                nc.vector.reciprocal(recip, den_of[h])
                oh = opool.tile([128, 512], BF16, name=f"oh{j}_{h}", tag=f"o{h}")
                nc.vector.tensor_mul(oh, pv_of[h], recip)
                out_h[h] = oh

        for t in range(min(LOOKAHEAD, n)):
            emit_score(t)
        for t in range(n):
            if t + LOOKAHEAD < n:
                emit_score(t + LOOKAHEAD)
            emit_pv(t)

    def wo_groups(j):
        """Closures: 16 wo psum groups (4 row-blocks x 4 col-slices, 4-head
        accumulation); the nn==3 group also DMAs the finished row-block."""
        groups = []
        ys_of = {}

        def group(tt, nn):
            def emit():
                if nn == 0:
                    ys_of[tt] = ypool.tile([128, D], BF16, name=f"ys{j}_{tt}", tag="ys")
                acc = pg.tile([128, 512], F32, name=f"wy{j}_{tt}_{nn}", tag="pg")
                for h in range(HPC):
                    nc.tensor.matmul(
                        acc,
                        out_h[h][:, 128 * tt:128 * (tt + 1)],
                        wo_sb[:, h, 512 * nn:512 * (nn + 1)],
                        start=(h == 0), stop=(h == HPC - 1),
                    )
                nc.vector.tensor_copy(
                    out=ys_of[tt][:, 512 * nn:512 * (nn + 1)], in_=acc
                )
                if nn == 3:
                    row = 512 * j + 128 * tt
                    st.dma_start(out=y[row:row + 128, :], in_=ys_of[tt])
            return emit

        for tt in range(4):
            for nn in range(4):
                groups.append(group(tt, nn))
        return groups

    # Steady state per chunk j: attn(j) -> proj(j+1) q/k chains (their long
    # RoPE drains never sit between two short psum groups) -> v chains of
    # proj(j+1) interleaved 1:4 with wo(j) groups (v's ACT-copy drain frees
    # its psum fast enough for the interleave).
    for c in proj_chains(0, xs, *cs0):
        c()
    for j in range(NSQ):
        if j + 1 < NSQ:
            xs_next = load_x(j + 1)
            cs_next = load_rope(j + 1)
        emit_attn(j)
        wo = wo_groups(j)
        if j + 1 < NSQ:
            chains = proj_chains(j + 1, xs_next, *cs_next)
            for c in chains[:8]:      # q/k chains
                c()
            for m in range(4):        # v chains, wo groups woven between
                chains[8 + m]()
                for g in wo[4 * m:4 * (m + 1)]:
                    g()
        else:
            for g in wo:
                g()

    for p in (psden, pspv, pss, pg, ypool, rpool, opool, ptpool, tpool,
              xpool, ropec, qpool, vpool, kpool, wpool, consts):
        p.release()


_PROGRAM = None


def build_program():
    global _PROGRAM
    if _PROGRAM is None:
        nc = bacc.Bacc("TRN2", target_bir_lowering=False, debug=False)
        with tile.TileContext(nc) as tc:
            _emit(tc)
        nc.compile()
        _PROGRAM = nc
    return _PROGRAM


def make_core_inputs(x, freqs_cos, freqs_sin, wq, wk, wv, wo):
    """Host-side sharding: returns list of 8 per-core input dicts."""
    import ml_dtypes

    bf16 = ml_dtypes.bfloat16
    x = np.asarray(x, dtype=np.float32)
    freqs_cos = np.asarray(freqs_cos, dtype=np.float32)
    freqs_sin = np.asarray(freqs_sin, dtype=np.float32)
    wq = np.asarray(wq, dtype=np.float32)
    wk = np.asarray(wk, dtype=np.float32)
    wv = np.asarray(wv, dtype=np.float32)
    wo = np.asarray(wo, dtype=np.float32)

    cosq = np.ascontiguousarray(np.repeat(freqs_cos.T, 2, axis=0))  # [128, S]
    sinq = np.ascontiguousarray(np.repeat(freqs_sin.T, 2, axis=0))
    sinq[0::2, :] *= -1.0  # even rows: -sin; odd rows: +sin

    skl = np.arange(128)[:, None]
    sql = np.arange(512)[None, :]
    negm = np.stack(
        [np.where(128 * m + skl <= sql, 0.0, -10000.0).astype(bf16) for m in range(4)],
        axis=1,
    )  # [128, 4, 512]

    ident = np.eye(128, dtype=bf16)
    onesd = np.ones((128, 128), dtype=bf16)
    xTs = [np.ascontiguousarray(x[b].T).astype(bf16) for b in range(B)]
    in_maps = []
    for c in range(N_CORES):
        b, g = divmod(c, CPB)
        hsl = slice(512 * g, 512 * (g + 1))
        in_maps.append(
            {
                "xT": xTs[b],
                "wqT": np.ascontiguousarray(wq[hsl, :].T).astype(bf16),
                "wkT": np.ascontiguousarray(wk[hsl, :].T).astype(bf16),
                "wvT": np.ascontiguousarray(wv[hsl, :].T).astype(bf16),
                "woT": np.ascontiguousarray(wo[:, hsl].T).astype(bf16),
                "cosq": cosq,
                "sinq": sinq,
                "negm": negm,
                "ident": ident,
                "onesd": onesd,
            }
        )
    return in_maps


def run(inputs, trace=False, **spmd_kwargs):
    """Run the SPMD kernel on 8 cores.  Returns (y_full, BassKernelResults)."""
    nc = build_program()
    in_maps = make_core_inputs(
        inputs["x"], inputs["freqs_cos"], inputs["freqs_sin"],
        inputs["wq"], inputs["wk"], inputs["wv"], inputs["wo"],
    )
    res = bass_utils.run_bass_kernel_spmd(
        nc, in_maps, list(range(N_CORES)), trace=trace, **spmd_kwargs
    )
    out = np.zeros((B, S, D), dtype=np.float32)
    for c in range(N_CORES):
        out[c // CPB] += np.asarray(res.results[c]["y"]).astype(np.float32)
    return out, res


def kernel(**inputs):
    out, _ = run(inputs, trace=False)
    return out


def simulate_core(core_idx, inputs):
    """CoreSim-validate a single core's program; returns its partial y."""
    from concourse.bass_interp import CoreSim

    nc = build_program()
    in_maps = make_core_inputs(
        inputs["x"], inputs["freqs_cos"], inputs["freqs_sin"],
        inputs["wq"], inputs["wk"], inputs["wv"], inputs["wo"],
    )
    sim = CoreSim(nc)
    for name, arr in in_maps[core_idx].items():
        sim.tensor(name)[:] = arr
    sim.simulate()
    return np.array(sim.tensor("y"))


# revision 21
# speedup vs baseline: 1.2493x; 1.1459x over previous
"""Trainium2 Bass kernel for nn_Attention (dense transformer block):
y = Attention(RoPE(x@wqT), RoPE(x@wkT), x@wvT, causal) @ woT

Sharding: 8 cores = 2 batches x 4 head-groups (tensor-parallel heads,
data-parallel batch).  Each core handles one batch and 4 of the 16 heads
(512 of the 2048 channels): column-shard of wq/wk/wv, row-shard of wo.
Each core emits a full-shape [S, D] partial of y; the host sums the 4
partials per batch.

Fused chunk pipeline (per core, SPMD): for each 512-row seq chunk j,
  proj(j):  q/k (+RoPE) and v for chunk j from one pass over x tiles
            (v reuses the same SBUF x tiles as stationary operands —
            x is streamed from DRAM exactly once, in bf16)
  attn(j):  causal attention for all 4 heads over keys 0..j, transposed
            scores (sT[sk,sq] = kT.T @ qT), exp on ACT, causal mask folded
            into the score psum via an identity-matmul of a -1e4 pattern,
            denominator via accumulating ones-matmul on the PE, p/v in
            bf16, software-pipelined with a 2-tile lookahead so the PE
            never waits on exp
  wo(j):    row-parallel wo partial for the chunk's 4 row-blocks, DMA'd
            out per row-block
Chunk j+1's x tiles prefetch during attn(j); all weights stay resident.
"""

import os
import sys

import numpy as np

for _p in ("/opt/trn_rl_repo", "/root/.axon_site/_ro/trn_rl_repo"):
    if os.path.isdir(_p) and _p not in sys.path:
        sys.path.insert(0, _p)

import concourse.bass as bass
import concourse.tile as tile
from concourse import bacc
from concourse import mybir
from concourse import bass_utils

B, S, D, H = 2, 2048, 2048, 16
HD = 128                 # head dim
HPC = 4                  # heads per core
CPB = 4                  # cores per batch
N_CORES = 8
NK = D // 128            # 16 contraction chunks
NSQ = S // 512           # 4 sq chunks of 512
NSK = S // 128           # 16 sk tiles of 128
SCALE = float(1.0 / np.sqrt(np.float32(HD)))

F32 = mybir.dt.float32
F32R = mybir.dt.float32r
BF16 = mybir.dt.bfloat16

EXP = mybir.ActivationFunctionType.Exp
SWAP_MASK = [i ^ 1 for i in range(32)]

# diag-tile mask pattern m: columns < 128*m fully masked; keep score width
# >= 256 so the fp32r moving operand stays at full rate
DIAG_OFF = {0: 0, 1: 128, 2: 256, 3: 256}
LOOKAHEAD = 3


def _emit(tc):
    nc = tc.nc

    xT = nc.dram_tensor("xT", [D, S], BF16, kind="ExternalInput").ap()
    wqT = nc.dram_tensor("wqT", [D, HPC * HD], BF16, kind="ExternalInput").ap()
    wkT = nc.dram_tensor("wkT", [D, HPC * HD], BF16, kind="ExternalInput").ap()
    wvT = nc.dram_tensor("wvT", [D, HPC * HD], BF16, kind="ExternalInput").ap()
    woT = nc.dram_tensor("woT", [HPC * HD, D], BF16, kind="ExternalInput").ap()
    cosq = nc.dram_tensor("cosq", [HD, S], F32, kind="ExternalInput").ap()
    sinq = nc.dram_tensor("sinq", [HD, S], F32, kind="ExternalInput").ap()
    mask01 = nc.dram_tensor("mask01", [128, 4, 512], BF16, kind="ExternalInput").ap()
    onesd = nc.dram_tensor("onesd", [128, 128], F32R, kind="ExternalInput").ap()
    y = nc.dram_tensor("y", [S, D], BF16, kind="ExternalOutput").ap()

    ld = nc.sync        # all loads on the SP HWDGE queue
    st = nc.scalar      # y stores on the ACT HWDGE queue

    # ---- SBUF pools (all live for the whole kernel)
    consts = tc.alloc_tile_pool(name="consts", bufs=1)
    mask_sb = consts.tile([128, 4, 512], BF16, name="mask_sb")
    ones_sb = consts.tile([128, 128], F32R, name="ones_sb")

    wpool = tc.alloc_tile_pool(name="wpool", bufs=1)
    wq_sb = wpool.tile([128, NK, HPC * HD], BF16, name="wq_sb")
    wk_sb = wpool.tile([128, NK, HPC * HD], BF16, name="wk_sb")
    wv_sb = wpool.tile([128, NK, HPC * HD], BF16, name="wv_sb")
    wo_sb = wpool.tile([128, HPC, D], BF16, name="wo_sb")

    kpool = tc.alloc_tile_pool(name="kpool", bufs=1)
    kT = [[kpool.tile([128, 512], F32R, name=f"kT{h}_{j}") for j in range(NSQ)]
          for h in range(HPC)]
    vpool = tc.alloc_tile_pool(name="vpool", bufs=1)
    v_sb = [vpool.tile([128, HPC * HD], BF16, name=f"v{m}") for m in range(NSK)]

    qpool = tc.alloc_tile_pool(name="qpool", bufs=1)
    ropec = tc.alloc_tile_pool(name="ropec", bufs=2)
    xpool = tc.alloc_tile_pool(name="xpool", bufs=2)
    tpool = tc.alloc_tile_pool(name="tpool", bufs=2)
    ptpool = tc.alloc_tile_pool(name="ptpool", bufs=6)
    dpool = tc.alloc_tile_pool(name="dpool", bufs=2)
    opool = tc.alloc_tile_pool(name="opool", bufs=1)
    rpool = tc.alloc_tile_pool(name="rpool", bufs=2)
    ypool = tc.alloc_tile_pool(name="ypool", bufs=2)

    pg = tc.alloc_tile_pool(name="pg", bufs=2, space="PSUM")
    pss = tc.alloc_tile_pool(name="pss", bufs=LOOKAHEAD + 1, space="PSUM")
    pspv = tc.alloc_tile_pool(name="pspv", bufs=2, space="PSUM")

    # ---- prologue DMAs; xs/wq split in halves so the first chain starts
    # after ~1/4 of the lead-in bytes.  Loads split over both HWDGE queues.
    def load_x(j):
        xt = xpool.tile([128, NK, 512], BF16, name=f"xs{j}", tag="xs")
        src = xT[:, 512 * j:512 * (j + 1)].rearrange("(kt p) c -> p kt c", p=128)
        ld.dma_start(out=xt[:, 0:NK // 2, :], in_=src[:, 0:NK // 2, :])
        ld.dma_start(out=xt[:, NK // 2:, :], in_=src[:, NK // 2:, :])
        return xt

    def load_rope(j):
        ct = ropec.tile([128, 512], F32, name=f"cos{j}", tag="cos")
        st.dma_start(out=ct, in_=cosq[:, 512 * j:512 * (j + 1)])
        sn = ropec.tile([128, 512], F32, name=f"sin{j}", tag="sin")
        st.dma_start(out=sn, in_=sinq[:, 512 * j:512 * (j + 1)])
        return ct, sn

    xs = load_x(0)
    wq_src = wqT.rearrange("(kt p) c -> p kt c", p=128)
    st.dma_start(out=wq_sb[:, 0:NK // 2, :], in_=wq_src[:, 0:NK // 2, :])
    st.dma_start(out=wq_sb[:, NK // 2:, :], in_=wq_src[:, NK // 2:, :])
    cs0 = load_rope(0)
    ld.dma_start(out=wk_sb, in_=wkT.rearrange("(kt p) c -> p kt c", p=128))
    ld.dma_start(out=wv_sb, in_=wvT.rearrange("(kt p) c -> p kt c", p=128))
    st.dma_start(out=mask_sb, in_=mask01)
    st.dma_start(out=ones_sb, in_=onesd)
    ld.dma_start(out=wo_sb, in_=woT.rearrange("(h p) d -> p h d", p=128))

    qT = [None] * HPC    # per-chunk q tiles, rewritten each chunk
    out_h = [None] * HPC

    def proj_chains(j, xs, cos_sb, sin_sb):
        """Closures: 8 q/k chains (+RoPE drain on DVE), 4 v chains (ACT
        drain).  The psum is freed by the 2nd DVE op (t2) for q/k, and by
        the single ACT copy for v."""
        chains = []

        def qk_chain(which, w_sb, h):
            def emit():
                acc = pg.tile([128, 512], F32, name=f"a{which}{j}_{h}", tag="pg")
                for k in range(NK):
                    nc.tensor.matmul(
                        acc, w_sb[:, k, 128 * h:128 * (h + 1)], xs[:, k, :],
                        start=(k == 0), stop=(k == NK - 1),
                    )
                if which == "q":
                    dst = qpool.tile([128, 512], F32R, name=f"qT{h}_{j}", tag=f"q{h}")
                    qT[h] = dst
                else:
                    dst = kT[h][j]
                shuf = tpool.tile([128, 512], F32, name=f"sh{which}{j}_{h}", tag="shuf")
                nc.vector.stream_shuffle(shuf, acc, SWAP_MASK)
                t2 = tpool.tile([128, 512], F32, name=f"t2{which}{j}_{h}", tag="t1")
                nc.vector.tensor_mul(t2, acc, cos_sb)
                t1 = tpool.tile([128, 512], F32, name=f"t1{which}{j}_{h}", tag="shuf")
                nc.vector.tensor_mul(t1, shuf, sin_sb)
                nc.vector.tensor_add(dst, t1, t2)
            return emit

        def v_chain(m):
            def emit():
                acc = pg.tile([128, HPC * HD], F32, name=f"av{j}_{m}", tag="pg")
                for k in range(NK):
                    nc.tensor.matmul(
                        acc, xs[:, k, 128 * m:128 * (m + 1)], wv_sb[:, k, :],
                        start=(k == 0), stop=(k == NK - 1),
                    )
                nc.scalar.copy(v_sb[4 * j + m], acc)
            return emit

        for which, w_sb in (("q", wq_sb), ("k", wk_sb)):
            for h in range(HPC):
                chains.append(qk_chain(which, w_sb, h))
        for m in range(4):
            chains.append(v_chain(m))
        return chains

    def emit_attn(j):
        """Causal attention for all heads over keys 0..4j+3, pipelined.
        Denominator = running sum of prob tiles on the DVE (per-head SBUF
        accumulator); causal mask = DVE multiply after the exp."""
        nsk = 4 * j + 4
        # flat tile list: per head, off-diagonal tiles first, then diagonal
        tiles = []
        for h in range(HPC):
            order = list(range(0, 4 * j)) + list(range(4 * j, nsk))
            for pos, i in enumerate(order):
                tiles.append((h, i, pos, pos == nsk - 1))
        n = len(tiles)
        st_of = {}   # tile idx -> (pt, off)
        pv_of = {}
        dacc_of = {}

        def emit_score(t):
            h, i, pos, last = tiles[t]
            off = DIAG_OFF[i - 4 * j] if i >= 4 * j else 0
            cs = slice(off, 512)
            s_ps = pss.tile([128, 512], F32, name=f"s{j}_{t}", tag="s")
            nc.tensor.matmul(
                s_ps[:, cs], kT[h][i // 4][:, 128 * (i % 4):128 * (i % 4 + 1)],
                qT[h][:, cs], start=True, stop=True,
            )
            pt = ptpool.tile([128, 512], BF16, name=f"p{j}_{t}", tag="pt")
            nc.scalar.activation(pt[:, cs], s_ps[:, cs], EXP, bias=0.0, scale=SCALE)
            if i >= 4 * j:
                nc.vector.tensor_mul(
                    pt[:, cs], pt[:, cs], mask_sb[:, i - 4 * j, cs]
                )
            # denominator running sum on DVE (pos 0 tile is always full-width)
            if pos == 0:
                dacc_of[h] = dpool.tile([128, 512], F32R, name=f"da{j}_{h}", tag="dacc")
                nc.vector.tensor_copy(out=dacc_of[h], in_=pt)
            else:
                nc.vector.tensor_add(dacc_of[h][:, cs], dacc_of[h][:, cs], pt[:, cs])
            st_of[t] = (pt, off)

        def emit_pv(t):
            h, i, pos, last = tiles[t]
            pt, off = st_of.pop(t)
            cs = slice(off, 512)
            if pos == 0:
                pv_of[h] = pspv.tile([128, 512], F32, name=f"pv{j}_{h}", tag="pv")
            nc.tensor.matmul(
                pv_of[h][:, cs], v_sb[i][:, 128 * h:128 * (h + 1)], pt[:, cs],
                start=(pos == 0), stop=last,
            )
            if last:
                den_ps = pss.tile([128, 512], F32, name=f"dn{j}_{h}", tag="s")
                nc.tensor.matmul(den_ps, ones_sb, dacc_of[h], start=True, stop=True)
                recip = rpool.tile([128, 512], F32, name=f"rc{j}_{h}", tag="recip")
                nc.vector.reciprocal(recip, den_ps)
                oh = opool.tile([128, 512], BF16, name=f"oh{j}_{h}", tag=f"o{h}")
                nc.vector.tensor_mul(oh, pv_of[h], recip)
                out_h[h] = oh

        for t in range(min(LOOKAHEAD, n)):
            emit_score(t)
        for t in range(n):
            if t + LOOKAHEAD < n:
                emit_score(t + LOOKAHEAD)
            emit_pv(t)

    def wo_groups(j):
        """Closures: 16 wo psum groups (4 row-blocks x 4 col-slices, 4-head
        accumulation); the nn==3 group also DMAs the finished row-block."""
        groups = []
        ys_of = {}

        def group(tt, nn):
            def emit():
                if nn == 0:
                    ys_of[tt] = ypool.tile([128, D], BF16, name=f"ys{j}_{tt}", tag="ys")
                acc = pg.tile([128, 512], F32, name=f"wy{j}_{tt}_{nn}", tag="pg")
                for h in range(HPC):
                    nc.tensor.matmul(
                        acc,
                        out_h[h][:, 128 * tt:128 * (tt + 1)],
                        wo_sb[:, h, 512 * nn:512 * (nn + 1)],
                        start=(h == 0), stop=(h == HPC - 1),
                    )
                dst = ys_of[tt][:, 512 * nn:512 * (nn + 1)]
                if nn % 2 == 0:
                    nc.vector.tensor_copy(out=dst, in_=acc)
                else:
                    nc.scalar.copy(dst, acc)
                if nn == 3:
                    row = 512 * j + 128 * tt
                    st.dma_start(out=y[row:row + 128, :], in_=ys_of[tt])
            return emit

        for tt in range(4):
            for nn in range(4):
                groups.append(group(tt, nn))
        return groups

    # Steady state per chunk j: attn(j) -> proj(j+1) q/k chains (their long
    # RoPE drains never sit between two short psum groups) -> v chains of
    # proj(j+1) interleaved 1:4 with wo(j) groups (v's ACT-copy drain frees
    # its psum fast enough for the interleave).
    for c in proj_chains(0, xs, *cs0):
        c()
    for j in range(NSQ):
        if j + 1 < NSQ:
            xs_next = load_x(j + 1)
            cs_next = load_rope(j + 1)
        emit_attn(j)
        wo = wo_groups(j)
        if j + 1 < NSQ:
            chains = proj_chains(j + 1, xs_next, *cs_next)
            for c in chains[:8]:      # q/k chains
                c()
            for m in range(4):        # v chains, wo groups woven between
                chains[8 + m]()
                for g in wo[4 * m:4 * (m + 1)]:
                    g()
        else:
            for g in wo:
                g()

    for p in (pspv, pss, pg, ypool, rpool, opool, dpool, ptpool, tpool,
              xpool, ropec, qpool, vpool, kpool, wpool, consts):
        p.release()


_PROGRAM = None


def build_program():
    global _PROGRAM
    if _PROGRAM is None:
        nc = bacc.Bacc("TRN2", target_bir_lowering=False, debug=False)
        with tile.TileContext(nc) as tc:
            _emit(tc)
        nc.compile()
        _PROGRAM = nc
    return _PROGRAM


def make_core_inputs(x, freqs_cos, freqs_sin, wq, wk, wv, wo):
    """Host-side sharding: returns list of 8 per-core input dicts."""
    import ml_dtypes

    bf16 = ml_dtypes.bfloat16
    x = np.asarray(x, dtype=np.float32)
    freqs_cos = np.asarray(freqs_cos, dtype=np.float32)
    freqs_sin = np.asarray(freqs_sin, dtype=np.float32)
    wq = np.asarray(wq, dtype=np.float32)
    wk = np.asarray(wk, dtype=np.float32)
    wv = np.asarray(wv, dtype=np.float32)
    wo = np.asarray(wo, dtype=np.float32)

    cosq = np.ascontiguousarray(np.repeat(freqs_cos.T, 2, axis=0))  # [128, S]
    sinq = np.ascontiguousarray(np.repeat(freqs_sin.T, 2, axis=0))
    sinq[0::2, :] *= -1.0  # even rows: -sin; odd rows: +sin

    skl = np.arange(128)[:, None]
    sql = np.arange(512)[None, :]
    mask01 = np.stack(
        [(128 * m + skl <= sql).astype(bf16) for m in range(4)], axis=1
    )  # [128, 4, 512]

    xTs = [np.ascontiguousarray(x[b].T).astype(bf16) for b in range(B)]
    in_maps = []
    for c in range(N_CORES):
        b, g = divmod(c, CPB)
        hsl = slice(512 * g, 512 * (g + 1))
        in_maps.append(
            {
                "xT": xTs[b],
                "wqT": np.ascontiguousarray(wq[hsl, :].T).astype(bf16),
                "wkT": np.ascontiguousarray(wk[hsl, :].T).astype(bf16),
                "wvT": np.ascontiguousarray(wv[hsl, :].T).astype(bf16),
                "woT": np.ascontiguousarray(wo[:, hsl].T).astype(bf16),
                "cosq": cosq,
                "sinq": sinq,
                "mask01": mask01,
                "onesd": np.ones((128, 128), dtype=np.float32),
            }
        )
    return in_maps


def run(inputs, trace=False, **spmd_kwargs):
    """Run the SPMD kernel on 8 cores.  Returns (y_full, BassKernelResults)."""
    nc = build_program()
    in_maps = make_core_inputs(
        inputs["x"], inputs["freqs_cos"], inputs["freqs_sin"],
        inputs["wq"], inputs["wk"], inputs["wv"], inputs["wo"],
    )
    res = bass_utils.run_bass_kernel_spmd(
        nc, in_maps, list(range(N_CORES)), trace=trace, **spmd_kwargs
    )
    out = np.zeros((B, S, D), dtype=np.float32)
    for c in range(N_CORES):
        out[c // CPB] += np.asarray(res.results[c]["y"]).astype(np.float32)
    return out, res


def kernel(**inputs):
    out, _ = run(inputs, trace=False)
    return out


def simulate_core(core_idx, inputs):
    """CoreSim-validate a single core's program; returns its partial y."""
    from concourse.bass_interp import CoreSim

    nc = build_program()
    in_maps = make_core_inputs(
        inputs["x"], inputs["freqs_cos"], inputs["freqs_sin"],
        inputs["wq"], inputs["wk"], inputs["wv"], inputs["wo"],
    )
    sim = CoreSim(nc)
    for name, arr in in_maps[core_idx].items():
        sim.tensor(name)[:] = arr
    sim.simulate()
    return np.array(sim.tensor("y"))
